# revision 6
# baseline (speedup 1.0000x reference)
"""Trainium2 Bass kernel for DynamicSparseAttention (B=4, C=256, H=W=64).

Sharding: 2 cores per batch element (8 cores total). Each core computes the
gate + K/V for its full batch element (tiny duplicated work) and handles
2048 of the 4096 queries. Everything stays channel-major so the attention
needs no transposes:
  - Q,K computed channel-major [C, N]; V token-major [N, C]
  - S^T tiles [keys=128, q=512] = K_tile.T @ Q  (keys on partitions)
  - softmax sums via ones-vector matmul on the PE (cross-partition reduce)
  - O^T [C, q] = V_tile.T @ exp(S^T), normalized by 1/sum (and the gate
    scalar) during the PSUM->SBUF copy
  - proj channel-major, residual add, DMA out
Matmuls in bf16 (fp32 PSUM accumulate): the attention path contributes only
~3% of the output magnitude (residual dominates), measured end-to-end error
vs the fp32 reference is ~1e-3 absmax-relative.
"""

import numpy as np
import ml_dtypes

import concourse.bass as bass
import concourse.bacc as bacc
import concourse.mybir as mybir
import concourse.tile as tile
from concourse.bass import ts
from concourse.bass_utils import run_bass_kernel_spmd

F32 = mybir.dt.float32
BF16 = mybir.dt.bfloat16
AF = mybir.ActivationFunctionType
ALU = mybir.AluOpType

B, C, H, W = 4, 256, 64, 64
N = H * W              # 4096 tokens per batch element
P = 128                # partitions
CT = C // P            # channel tiles (2)
NCORES = 8
QN = N * B // NCORES   # queries per core (2048)
HID = 32
QCH = 512              # query chunk for attention
MT = N // P            # key tiles (32)
NQC = QN // QCH        # query chunks per core (4)
SCALE = 1.0 / np.sqrt(C)

bf16 = ml_dtypes.bfloat16


def _build():
    nc = bacc.Bacc()

    xfull = nc.declare_dram_parameter("xfull", [C, N], F32, isOutput=False)
    xq = nc.declare_dram_parameter("xq", [C, QN], BF16, isOutput=False)
    xres = nc.declare_dram_parameter("xres", [C, QN], F32, isOutput=False)
    wqT = nc.declare_dram_parameter("wqT", [C, C], BF16, isOutput=False)
    wkT = nc.declare_dram_parameter("wkT", [C, C], BF16, isOutput=False)
    wvT = nc.declare_dram_parameter("wvT", [C, C], BF16, isOutput=False)
    wpT = nc.declare_dram_parameter("wpT", [C, C], BF16, isOutput=False)
    wce1T = nc.declare_dram_parameter("wce1T", [C, HID], F32, isOutput=False)
    wce2T = nc.declare_dram_parameter("wce2T", [HID, 1], F32, isOutput=False)
    out = nc.declare_dram_parameter("out", [C, QN], F32, isOutput=True)

    # [ (t p) n ] -> [p t n] views for 128-partition SBUF tiles
    xfull_r = xfull.rearrange("(t p) n -> p t n", p=P)
    xq_r = xq.rearrange("(t p) n -> p t n", p=P)
    xres_r = xres.rearrange("(t p) n -> p t n", p=P)
    wq_r = wqT.rearrange("(t p) o -> p t o", p=P)
    wk_r = wkT.rearrange("(t p) o -> p t o", p=P)
    wv_r = wvT.rearrange("(t p) o -> p t o", p=P)
    wp_r = wpT.rearrange("(t p) o -> p t o", p=P)
    wce1_r = wce1T.rearrange("(t p) h -> p t h", p=P)
    out_r = out.rearrange("(t p) n -> t p n", p=P)

    with tile.TileContext(nc) as tc:
        with (
            tc.tile_pool(name="cst", bufs=1) as cst,
            tc.tile_pool(name="ework", bufs=1) as ework,
            tc.tile_pool(name="work", bufs=2) as work,
            tc.tile_pool(name="ps", bufs=1, space="PSUM") as psum,
        ):
            # ---- loads ----
            wq_sb = cst.tile([P, CT, C], BF16)
            nc.sync.dma_start(wq_sb[:], wq_r[:])
            wk_sb = cst.tile([P, CT, C], BF16)
            nc.sync.dma_start(wk_sb[:], wk_r[:])
            wv_sb = cst.tile([P, CT, C], BF16)
            nc.sync.dma_start(wv_sb[:], wv_r[:])
            wp_sb = cst.tile([P, CT, C], BF16)
            nc.sync.dma_start(wp_sb[:], wp_r[:])
            wce1_sb = cst.tile([P, CT, HID], F32)
            nc.sync.dma_start(wce1_sb[:], wce1_r[:])
            wce2_sb = cst.tile([HID, 1], F32)
            nc.sync.dma_start(wce2_sb[:], wce2T[:])

            x_sb = cst.tile([P, CT, N], F32)
            xq_sb = cst.tile([P, CT, QN], BF16)
            for t in range(CT):
                nc.sync.dma_start(x_sb[:, t, :], xfull_r[:, t, :])
                nc.sync.dma_start(xq_sb[:, t, :], xq_r[:, t, :])

            ones_bf = cst.tile([P, 1], BF16)
            nc.vector.memset(ones_bf[:], 1.0)
            onesrow = cst.tile([1, P], F32)
            nc.vector.memset(onesrow[:], 1.0)

            # ---- cast x to bf16 (ACT) + feat = mean+max (DVE) ----
            xb_sb = cst.tile([P, CT, N], BF16)
            xsum = cst.tile([P, CT], F32)
            xmax8 = cst.tile([P, CT, 8], F32)
            feat = cst.tile([P, CT], F32)
            for t in range(CT):
                nc.scalar.activation(
                    xb_sb[:, t, :], x_sb[:, t, :], AF.Copy,
                    accum_out=xsum[:, t:t + 1],
                )
                for j in range(8):
                    nc.vector.reduce_max(
                        xmax8[:, t, j:j + 1], x_sb[:, t, ts(j, N // 8)],
                        axis=mybir.AxisListType.X,
                    )
            for t in range(CT):
                # feat = xsum/N + max(xmax8)
                xm = work.tile([P, 1], F32, tag="xm")
                nc.vector.reduce_max(
                    xm[:], xmax8[:, t, :], axis=mybir.AxisListType.X
                )
                nc.vector.scalar_tensor_tensor(
                    feat[:, t:t + 1], xsum[:, t:t + 1], 1.0 / N, xm[:],
                    op0=ALU.mult, op1=ALU.add,
                )

            # ---- gate MLP (tiny) ----
            ph = psum.tile([HID, 1], F32, tag="gate", bufs=1)
            for t in range(CT):
                nc.tensor.matmul(
                    ph[:], wce1_sb[:, t, :], feat[:, t:t + 1],
                    start=(t == 0), stop=(t == CT - 1),
                )
            sig = cst.tile([HID, 1], F32)
            nc.scalar.activation(sig[:], ph[:], AF.Sigmoid)
            hid_sb = cst.tile([HID, 1], F32)
            nc.vector.tensor_tensor(hid_sb[:], ph[:], sig[:], op=ALU.mult)
            pc = psum.tile([1, 1], F32, tag="gate")
            nc.tensor.matmul(pc[:], wce2_sb[:], hid_sb[:])
            cmplx = cst.tile([1, 1], F32)
            nc.scalar.activation(cmplx[:], pc[:], AF.Sigmoid)

            # ---- K, Q (channel-major), V (token-major) ----
            k_sb = cst.tile([P, CT, N], BF16)
            q_sb = cst.tile([P, CT, QN], BF16)
            v_sb = cst.tile([P, MT, C], BF16)
            for t in range(CT):
                for j in range(N // 512):
                    pk = psum.tile([P, 512], F32, tag="pj", bufs=2)
                    for kc in range(CT):
                        nc.tensor.matmul(
                            pk[:], wk_sb[:, kc, ts(t, P)],
                            xb_sb[:, kc, ts(j, 512)],
                            start=(kc == 0), stop=(kc == CT - 1),
                        )
                    nc.vector.tensor_copy(k_sb[:, t, ts(j, 512)], pk[:])
            for t in range(CT):
                for j in range(QN // 512):
                    pq = psum.tile([P, 512], F32, tag="pj", bufs=2)
                    for kc in range(CT):
                        nc.tensor.matmul(
                            pq[:], wq_sb[:, kc, ts(t, P)],
                            xq_sb[:, kc, ts(j, 512)],
                            start=(kc == 0), stop=(kc == CT - 1),
                        )
                    nc.vector.tensor_copy(q_sb[:, t, ts(j, 512)], pq[:])
            for nt in range(MT):
                pv = psum.tile([P, C], F32, tag="pj", bufs=2)
                for kc in range(CT):
                    nc.tensor.matmul(
                        pv[:], xb_sb[:, kc, ts(nt, P)], wv_sb[:, kc, :],
                        start=(kc == 0), stop=(kc == CT - 1),
                    )
                nc.scalar.activation(v_sb[:, nt, :], pv[:], AF.Copy)

            # ---- attention ----
            o_sb = cst.tile([P, CT, QN], BF16)
            for qc in range(NQC):
                et = ework.tile([P, MT, QCH], BF16, tag="exp")
                pr = psum.tile([1, QCH], F32, tag="pr", bufs=1)
                for mt in range(MT):
                    pst = psum.tile([P, QCH], F32, tag="ps", bufs=2)
                    for kc in range(CT):
                        nc.tensor.matmul(
                            pst[:], k_sb[:, kc, ts(mt, P)],
                            q_sb[:, kc, ts(qc, QCH)],
                            start=(kc == 0), stop=(kc == CT - 1),
                        )
                    nc.scalar.activation(et[:, mt, :], pst[:], AF.Exp)
                    nc.tensor.matmul(
                        pr[:], ones_bf[:], et[:, mt, :],
                        start=(mt == 0), stop=(mt == MT - 1),
                        skip_group_check=True,
                    )
                # 1/sum * gate, broadcast to 128 partitions via PE
                rr = work.tile([1, QCH], F32, tag="rr")
                nc.vector.reciprocal(rr[:], pr[:])
                rr2 = work.tile([1, QCH], F32, tag="rr2")
                nc.vector.tensor_scalar_mul(rr2[:], rr[:], cmplx[:1, :1])
                pb = psum.tile([P, QCH], F32, tag="ps", bufs=2)
                nc.tensor.matmul(pb[:], onesrow[:], rr2[:])
                recipB = work.tile([P, QCH], F32, tag="recipB", bufs=1)
                nc.scalar.activation(recipB[:], pb[:], AF.Copy)
                for ct in range(CT):
                    po = psum.tile([P, QCH], F32, tag="po", bufs=2)
                    for mt in range(MT):
                        nc.tensor.matmul(
                            po[:], v_sb[:, mt, ts(ct, P)], et[:, mt, :],
                            start=(mt == 0), stop=(mt == MT - 1),
                        )
                    # o = (po * recipB) -> bf16 (normalization + gate folded)
                    nc.vector.tensor_tensor(
                        o_sb[:, ct, ts(qc, QCH)], po[:], recipB[:],
                        op=ALU.mult,
                    )

            # ---- projection + residual ----
            for ct in range(CT):
                for j in range(QN // 512):
                    pp = psum.tile([P, 512], F32, tag="po", bufs=2)
                    for kc in range(CT):
                        nc.tensor.matmul(
                            pp[:], wp_sb[:, kc, ts(ct, P)],
                            o_sb[:, kc, ts(j, 512)],
                            start=(kc == 0), stop=(kc == CT - 1),
                        )
                    xr = work.tile([P, 512], F32, tag="xr", bufs=3)
                    nc.sync.dma_start(xr[:], xres_r[:, ct, ts(j, 512)])
                    outt = work.tile([P, 512], F32, tag="outt", bufs=3)
                    nc.vector.tensor_tensor(outt[:], pp[:], xr[:], op=ALU.add)
                    nc.sync.dma_start(out_r[ct, :, ts(j, 512)], outt[:])

    nc.finalize()
    return nc


_NC_CACHE = {}


def _get_nc():
    if "nc" not in _NC_CACHE:
        _NC_CACHE["nc"] = _build()
    return _NC_CACHE["nc"]


def kernel(x, w_ce1, w_ce2, wq, wk, wv, wproj):
    x = np.asarray(x, dtype=np.float32)
    assert x.shape == (B, C, H, W)

    wqT = np.ascontiguousarray(np.asarray(wq, np.float32).T * SCALE).astype(bf16)
    wkT = np.ascontiguousarray(np.asarray(wk, np.float32).T).astype(bf16)
    wvT = np.ascontiguousarray(np.asarray(wv, np.float32).T).astype(bf16)
    wpT = np.ascontiguousarray(np.asarray(wproj, np.float32).T).astype(bf16)
    wce1T = np.ascontiguousarray(np.asarray(w_ce1, np.float32).T)
    wce2T = np.ascontiguousarray(np.asarray(w_ce2, np.float32).T)

    in_maps = []
    for c in range(NCORES):
        b, h = divmod(c, NCORES // B)
        xf = np.ascontiguousarray(x[b].reshape(C, N))
        sl = slice(h * QN, (h + 1) * QN)
        in_maps.append({
            "xfull": xf,
            "xq": np.ascontiguousarray(xf[:, sl]).astype(bf16),
            "xres": np.ascontiguousarray(xf[:, sl]),
            "wqT": wqT, "wkT": wkT, "wvT": wvT, "wpT": wpT,
            "wce1T": wce1T, "wce2T": wce2T,
        })

    res = run_bass_kernel_spmd(_get_nc(), in_maps, list(range(NCORES)))

    out = np.empty((B, C, N), dtype=np.float32)
    for c in range(NCORES):
        b, h = divmod(c, NCORES // B)
        out[b][:, h * QN:(h + 1) * QN] = res.results[c]["out"]
    return out.reshape(B, C, H, W)


# revision 22
# speedup vs baseline: 1.1974x; 1.1974x over previous
"""Trainium2 Bass kernel for DynamicSparseAttention (B=4, C=256, H=W=64).

Sharding: 2 cores per batch element (8 cores total). Each core computes the
gate + K/V for its full batch element (tiny duplicated work) and handles
2048 of the 4096 queries. Everything stays channel-major so the attention
needs no transposes:
  - Q,K computed channel-major [C, N]; V token-major [N, C]
  - S^T tiles [keys=128, q=512] = K_tile.T @ Q  (keys on partitions)
  - softmax sums via ones-vector matmul on the PE (cross-partition reduce)
  - O^T [C, q] = V_tile.T @ exp(S^T), normalized by 1/sum (and the gate
    scalar) during the PSUM->SBUF copy
  - proj channel-major, residual add, DMA out
Matmuls in bf16 (fp32 PSUM accumulate): the attention path contributes only
~3% of the output magnitude (residual dominates), measured end-to-end error
vs the fp32 reference is ~1e-3 absmax-relative.
"""

import numpy as np
import ml_dtypes

import concourse.bass as bass
import concourse.bacc as bacc
import concourse.mybir as mybir
import concourse.tile as tile
from concourse.bass import ts
from concourse.bass_utils import run_bass_kernel_spmd

F32 = mybir.dt.float32
BF16 = mybir.dt.bfloat16
AF = mybir.ActivationFunctionType
ALU = mybir.AluOpType

B, C, H, W = 4, 256, 64, 64
N = H * W              # 4096 tokens per batch element
P = 128                # partitions
CT = C // P            # channel tiles (2)
NCORES = 8
QN = N * B // NCORES   # queries per core (2048)
HID = 32
QCH = 512              # query chunk for attention
MT = N // P            # key tiles (32)
NQC = QN // QCH        # query chunks per core (4)
SCALE = 1.0 / np.sqrt(C)

bf16 = ml_dtypes.bfloat16


def _build(reps=1, variant="full"):
    # variant flags for timing experiments (graded path always uses "full")
    no_sum = "nosum" in variant
    no_av = "noav" in variant
    no_attn = "noattn" in variant
    exp_pair = "exppair" in variant
    et_bufs = 1 if "et1" in variant else 2
    from contextlib import ExitStack

    nc = bacc.Bacc()

    xfull = nc.declare_dram_parameter("xfull", [C, N], F32, isOutput=False)
    xq = nc.declare_dram_parameter("xq", [C, QN], BF16, isOutput=False)
    xres = nc.declare_dram_parameter("xres", [C, QN], F32, isOutput=False)
    wqT = nc.declare_dram_parameter("wqT", [C, C], BF16, isOutput=False)
    wkT = nc.declare_dram_parameter("wkT", [C, C], BF16, isOutput=False)
    wvT = nc.declare_dram_parameter("wvT", [C, C], BF16, isOutput=False)
    wpT = nc.declare_dram_parameter("wpT", [C, C], BF16, isOutput=False)
    wce1T = nc.declare_dram_parameter("wce1T", [C, HID], F32, isOutput=False)
    wce2T = nc.declare_dram_parameter("wce2T", [HID, 1], F32, isOutput=False)
    out = nc.declare_dram_parameter("out", [C, QN], F32, isOutput=True)

    # [ (t p) n ] -> [p t n] views for 128-partition SBUF tiles
    xfull_r = xfull.rearrange("(t p) n -> p t n", p=P)
    xq_r = xq.rearrange("(t p) n -> p t n", p=P)
    xres_r = xres.rearrange("(t p) n -> p t n", p=P)
    wq_r = wqT.rearrange("(t p) o -> p t o", p=P)
    wk_r = wkT.rearrange("(t p) o -> p t o", p=P)
    wv_r = wvT.rearrange("(t p) o -> p t o", p=P)
    wp_r = wpT.rearrange("(t p) o -> p t o", p=P)
    wce1_r = wce1T.rearrange("(t p) h -> p t h", p=P)
    out_r = out.rearrange("(t p) n -> t p n", p=P)

    with tile.TileContext(nc) as tc:
        with (
            tc.tile_pool(name="cst", bufs=1) as cst,
            tc.tile_pool(name="ework", bufs=1) as ework,
            tc.tile_pool(name="work", bufs=2) as work,
            tc.tile_pool(name="ps", bufs=1, space="PSUM") as psum,
        ):
            _loop = ExitStack()
            if reps > 1:
                _loop.enter_context(tc.For_i(0, reps))
            # ---- loads ----
            wq_sb = cst.tile([P, CT, C], BF16)
            nc.sync.dma_start(wq_sb[:], wq_r[:])
            wk_sb = cst.tile([P, CT, C], BF16)
            nc.sync.dma_start(wk_sb[:], wk_r[:])
            wv_sb = cst.tile([P, CT, C], BF16)
            nc.sync.dma_start(wv_sb[:], wv_r[:])
            wp_sb = cst.tile([P, CT, C], BF16)
            nc.sync.dma_start(wp_sb[:], wp_r[:])
            wce1_sb = cst.tile([P, CT, HID], F32)
            nc.sync.dma_start(wce1_sb[:], wce1_r[:])
            wce2_sb = cst.tile([HID, 1], F32)
            nc.sync.dma_start(wce2_sb[:], wce2T[:])

            xq_sb = cst.tile([P, CT, QN], BF16)
            for j in range(QN // 512):
                for t in range(CT):
                    nc.sync.dma_start(
                        xq_sb[:, t, ts(j, 512)], xq_r[:, t, ts(j, 512)]
                    )

            ones_bf = cst.tile([P, 1], BF16)
            nc.vector.memset(ones_bf[:], 1.0)
            onesrow = cst.tile([1, P], BF16)
            nc.vector.memset(onesrow[:], 1.0)

            # ---- stream x in chunks: cast to bf16 (ACT) + feat stats ----
            NXC = 8  # x chunks per c-tile
            XCH = N // NXC
            xb_sb = cst.tile([P, CT, N], BF16)
            xsum8 = cst.tile([P, CT, NXC], F32)
            xmax8 = cst.tile([P, CT, NXC], F32)
            feat = cst.tile([P, CT], F32)
            for j in range(NXC):
                for t in range(CT):
                    xc = work.tile([P, XCH], F32, tag="xc", bufs=3)
                    nc.sync.dma_start(xc[:], xfull_r[:, t, ts(j, XCH)])
                    nc.scalar.activation(
                        xb_sb[:, t, ts(j, XCH)], xc[:], AF.Copy,
                        accum_out=xsum8[:, t, j:j + 1],
                    )
                    nc.vector.reduce_max(
                        xmax8[:, t, j:j + 1], xc[:],
                        axis=mybir.AxisListType.X,
                    )
            for t in range(CT):
                # feat = sum(xsum8)/N + max(xmax8)
                xm = work.tile([P, 1], F32, tag="xm")
                nc.vector.reduce_max(
                    xm[:], xmax8[:, t, :], axis=mybir.AxisListType.X
                )
                xs = work.tile([P, 1], F32, tag="xs")
                nc.vector.reduce_sum(
                    xs[:], xsum8[:, t, :], axis=mybir.AxisListType.X
                )
                nc.vector.scalar_tensor_tensor(
                    feat[:, t:t + 1], xs[:], 1.0 / N, xm[:],
                    op0=ALU.mult, op1=ALU.add,
                )

            # ---- gate MLP (tiny) ----
            ph = psum.tile([HID, 1], F32, tag="pr", bufs=1)
            for t in range(CT):
                nc.tensor.matmul(
                    ph[:], wce1_sb[:, t, :], feat[:, t:t + 1],
                    start=(t == 0), stop=(t == CT - 1),
                )
            sig = cst.tile([HID, 1], F32)
            nc.scalar.activation(sig[:], ph[:], AF.Sigmoid)
            hid_sb = cst.tile([HID, 1], F32)
            nc.vector.tensor_tensor(hid_sb[:], ph[:], sig[:], op=ALU.mult)
            pc = psum.tile([1, 1], F32, tag="pr")
            nc.tensor.matmul(pc[:], wce2_sb[:], hid_sb[:])
            cmplx = cst.tile([1, 1], F32)
            nc.scalar.activation(cmplx[:], pc[:], AF.Sigmoid)

            # ---- Q, K (channel-major), V (token-major) ----
            k_sb = cst.tile([P, CT, N], BF16)
            q_sb = cst.tile([P, CT, QN], BF16)
            v_sb = cst.tile([P, MT, C], BF16)
            for j in range(QN // 512):
                for t in range(CT):
                    pq = psum.tile([P, 512], F32, tag="po", bufs=2)
                    for kc in range(CT):
                        nc.tensor.matmul(
                            pq[:], wq_sb[:, kc, ts(t, P)],
                            xq_sb[:, kc, ts(j, 512)],
                            start=(kc == 0), stop=(kc == CT - 1),
                        )
                    nc.vector.tensor_copy(q_sb[:, t, ts(j, 512)], pq[:])
            for j in range(N // 512):
                for t in range(CT):
                    pk = psum.tile([P, 512], F32, tag="po", bufs=2)
                    for kc in range(CT):
                        nc.tensor.matmul(
                            pk[:], wk_sb[:, kc, ts(t, P)],
                            xb_sb[:, kc, ts(j, 512)],
                            start=(kc == 0), stop=(kc == CT - 1),
                        )
                    nc.vector.tensor_copy(k_sb[:, t, ts(j, 512)], pk[:])

            for np_ in range(MT // 2):
                pv2 = psum.tile([P, 2, C], F32, tag="po", bufs=2)
                for h in range(2):
                    nt = 2 * np_ + h
                    for kc in range(CT):
                        nc.tensor.matmul(
                            pv2[:, h, :], xb_sb[:, kc, ts(nt, P)],
                            wv_sb[:, kc, :],
                            start=(kc == 0), stop=(kc == CT - 1),
                        )
                nc.scalar.activation(
                    v_sb[:, 2 * np_:2 * np_ + 2, :], pv2[:], AF.Copy
                )

            # ---- attention + per-chunk projection/residual ----
            o_sb = cst.tile([P, CT, QN], BF16)
            for qc in range(NQC if not no_attn else 0):
                et = ework.tile([P, MT, QCH], BF16, tag="exp", bufs=et_bufs)
                pr = psum.tile([1, QCH], F32, tag="pr", bufs=1)
                # streaming pairwise tree-sum of exp tiles on DVE (bf16 adds
                # are cheap and keep the PE free); one ones-matmul at the end
                # does the cross-partition reduction.
                tree_stack = []

                def tree_push(leaf):
                    cur = (0, leaf)
                    while tree_stack and tree_stack[-1][0] == cur[0]:
                        lv, prev = tree_stack.pop()
                        dst = work.tile([P, QCH], BF16, tag="tree", bufs=7)
                        nc.vector.tensor_tensor(dst[:], prev, cur[1], op=ALU.add)
                        cur = (lv + 1, dst[:])
                    tree_stack.append(cur)

                if exp_pair:
                    for mp in range(MT // 2):
                        # two m-tiles share one 2-bank psum so a single exp op
                        # covers [128, 1024]
                        pst2 = psum.tile([P, 2, QCH], F32, tag="ps", bufs=2)
                        for h in range(2):
                            mt = 2 * mp + h
                            for kc in range(CT):
                                nc.tensor.matmul(
                                    pst2[:, h, :], k_sb[:, kc, ts(mt, P)],
                                    q_sb[:, kc, ts(qc, QCH)],
                                    start=(kc == 0), stop=(kc == CT - 1),
                                )
                        nc.scalar.activation(
                            et[:, 2 * mp:2 * mp + 2, :], pst2[:], AF.Exp
                        )
                        if not no_sum:
                            tree_push(et[:, 2 * mp, :])
                            tree_push(et[:, 2 * mp + 1, :])
                else:
                    po_list = []
                    if not no_av:
                        for ct in range(CT):
                            po = psum.tile(
                                [P, QCH], F32, tag="po", bufs=2, name=f"po{ct}"
                            )
                            po_list.append(po)
                    for mt in range(MT):
                        pst = psum.tile([P, QCH], F32, tag="ps", bufs=5)
                        for kc in range(CT):
                            nc.tensor.matmul(
                                pst[:], k_sb[:, kc, ts(mt, P)],
                                q_sb[:, kc, ts(qc, QCH)],
                                start=(kc == 0), stop=(kc == CT - 1),
                            )
                        nc.scalar.activation(et[:, mt, :], pst[:], AF.Exp)
                        # AV for this m-tile rides right behind the exp so the
                        # PE hides the ACT latency instead of stalling on it
                        for ct, po in enumerate(po_list):
                            nc.tensor.matmul(
                                po[:], v_sb[:, mt, ts(ct, P)], et[:, mt, :],
                                start=(mt == 0), stop=(mt == MT - 1),
                                skip_group_check=True,
                            )
                        if not no_sum:
                            tree_push(et[:, mt, :])
                if not no_sum:
                    assert len(tree_stack) == 1
                    nc.tensor.matmul(pr[:], ones_bf[:], tree_stack[-1][1])
                # 1/sum * gate, broadcast to 128 partitions via PE
                if not no_sum:
                    rr = work.tile([1, QCH], F32, tag="rr")
                    nc.vector.reciprocal(rr[:], pr[:])
                    rr2 = work.tile([1, QCH], BF16, tag="rr2")
                    nc.vector.tensor_scalar_mul(rr2[:], rr[:], cmplx[:1, :1])
                    pb = psum.tile([P, QCH], F32, tag="pr", bufs=1)
                    nc.tensor.matmul(pb[:], onesrow[:], rr2[:])
                    recipB = work.tile([P, QCH], F32, tag="recipB", bufs=1)
                    nc.scalar.activation(recipB[:], pb[:], AF.Copy)
                for ct in range(CT if not no_av else 0):
                    po = po_list[ct]
                    # o = (po * recipB) -> bf16 (normalization + gate folded)
                    if no_sum:
                        nc.vector.tensor_copy(o_sb[:, ct, ts(qc, QCH)], po[:])
                    else:
                        nc.vector.tensor_tensor(
                            o_sb[:, ct, ts(qc, QCH)], po[:], recipB[:],
                            op=ALU.mult,
                        )
                # projection + residual for this chunk
                for ct in range(CT if not no_av else 0):
                    pp = psum.tile([P, QCH], F32, tag="po", bufs=2)
                    for kc in range(CT):
                        nc.tensor.matmul(
                            pp[:], wp_sb[:, kc, ts(ct, P)],
                            o_sb[:, kc, ts(qc, QCH)],
                            start=(kc == 0), stop=(kc == CT - 1),
                        )
                    xr = work.tile([P, QCH], F32, tag="xr", bufs=3)
                    nc.sync.dma_start(xr[:], xres_r[:, ct, ts(qc, QCH)])
                    outt = work.tile([P, QCH], F32, tag="outt", bufs=3)
                    nc.vector.tensor_tensor(outt[:], pp[:], xr[:], op=ALU.add)
                    nc.sync.dma_start(out_r[ct, :, ts(qc, QCH)], outt[:])
            _loop.close()

    nc.finalize()
    return nc


_NC_CACHE = {}


def _get_nc():
    if "nc" not in _NC_CACHE:
        _NC_CACHE["nc"] = _build()
    return _NC_CACHE["nc"]


def kernel(x, w_ce1, w_ce2, wq, wk, wv, wproj):
    x = np.asarray(x, dtype=np.float32)
    assert x.shape == (B, C, H, W)

    wqT = np.ascontiguousarray(np.asarray(wq, np.float32).T * SCALE).astype(bf16)
    wkT = np.ascontiguousarray(np.asarray(wk, np.float32).T).astype(bf16)
    wvT = np.ascontiguousarray(np.asarray(wv, np.float32).T).astype(bf16)
    wpT = np.ascontiguousarray(np.asarray(wproj, np.float32).T).astype(bf16)
    wce1T = np.ascontiguousarray(np.asarray(w_ce1, np.float32).T)
    wce2T = np.ascontiguousarray(np.asarray(w_ce2, np.float32).T)

    in_maps = []
    for c in range(NCORES):
        b, h = divmod(c, NCORES // B)
        xf = np.ascontiguousarray(x[b].reshape(C, N))
        sl = slice(h * QN, (h + 1) * QN)
        in_maps.append({
            "xfull": xf,
            "xq": np.ascontiguousarray(xf[:, sl]).astype(bf16),
            "xres": np.ascontiguousarray(xf[:, sl]),
            "wqT": wqT, "wkT": wkT, "wvT": wvT, "wpT": wpT,
            "wce1T": wce1T, "wce2T": wce2T,
        })

    res = run_bass_kernel_spmd(_get_nc(), in_maps, list(range(NCORES)))

    out = np.empty((B, C, N), dtype=np.float32)
    for c in range(NCORES):
        b, h = divmod(c, NCORES // B)
        out[b][:, h * QN:(h + 1) * QN] = res.results[c]["out"]
    return out.reshape(B, C, H, W)


# revision 38
# speedup vs baseline: 7718.2030x; 6445.5616x over previous
"""Trainium2 Bass kernel for DynamicSparseAttention (B=4, C=256, H=W=64).

Sharding: 2 cores per batch element (8 cores total). Each core computes the
gate + K/V for its full batch element (tiny duplicated work) and handles
2048 of the 4096 queries. Everything stays channel-major so the attention
needs no transposes:
  - Q,K computed channel-major [C, N]; V token-major [N, C]
  - S^T tiles [keys=128, q=512] = K_tile.T @ Q  (keys on partitions)
  - softmax sums: pairwise DVE tree over exp tiles + one ones-matmul for
    the cross-partition reduction (no max subtraction needed: |logits|<~6)
  - O^T [C, q] = V_tile.T @ exp(S^T); AV matmuls ride right behind each
    exp so the PE hides the ACT latency; normalization by 1/sum and the
    gate scalar is folded into the PSUM->SBUF copy of O
  - proj channel-major, residual add, DMA out
Matmuls in bf16 (fp32 PSUM accumulate): the attention path contributes only
~3% of the output magnitude (residual dominates), measured end-to-end error
vs the fp32 reference is ~1e-3 absmax-relative.
"""

import numpy as np
import ml_dtypes

import concourse.bass as bass
import concourse.bacc as bacc
import concourse.mybir as mybir
import concourse.tile as tile
from concourse.bass import ts
from concourse.bass_utils import run_bass_kernel_spmd

F32 = mybir.dt.float32
BF16 = mybir.dt.bfloat16
AF = mybir.ActivationFunctionType
ALU = mybir.AluOpType

B, C, H, W = 4, 256, 64, 64
N = H * W              # 4096 tokens per batch element
P = 128                # partitions
CT = C // P            # channel tiles (2)
NCORES = 8
QN = N * B // NCORES   # queries per core (2048)
HID = 32
QCH = 512              # query chunk for attention
MT = N // P            # key tiles (32)
NQC = QN // QCH        # query chunks per core (4)
SCALE = 1.0 / np.sqrt(C)

bf16 = ml_dtypes.bfloat16


def _build(reps=1, variant="full"):
    # variant flags for timing experiments (graded path always uses "full")
    no_sum = "nosum" in variant
    no_av = "noav" in variant
    no_attn = "noattn" in variant
    exp_pair = "expsingle" not in variant
    exp_quad = "expquad" in variant
    et_bufs = 1 if "et1" in variant else 2
    pr_bufs = 1 if "pr1" in variant else 2
    dma2 = "dma1" not in variant
    tree_eng = "gpsimd" if "gpstree" in variant else "vector"
    max_eng = "gpsimd" if "gpsmax" in variant else "vector"
    from contextlib import ExitStack

    nc = bacc.Bacc()

    xq = nc.declare_dram_parameter("xq", [C, QN], BF16, isOutput=False)
    xres = nc.declare_dram_parameter("xres", [C, QN], F32, isOutput=False)
    xoth = nc.declare_dram_parameter("xoth", [C, QN], F32, isOutput=False)
    wqT = nc.declare_dram_parameter("wqT", [C, C], BF16, isOutput=False)
    wkT = nc.declare_dram_parameter("wkT", [C, C], BF16, isOutput=False)
    wvT = nc.declare_dram_parameter("wvT", [C, C], BF16, isOutput=False)
    wpT = nc.declare_dram_parameter("wpT", [C, C], BF16, isOutput=False)
    wce1T = nc.declare_dram_parameter("wce1T", [C, HID], F32, isOutput=False)
    wce2T = nc.declare_dram_parameter("wce2T", [HID, 1], F32, isOutput=False)
    out = nc.declare_dram_parameter("out", [C, QN], F32, isOutput=True)

    # [ (t p) n ] -> [p t n] views for 128-partition SBUF tiles
    xq_r = xq.rearrange("(t p) n -> p t n", p=P)
    xres_r = xres.rearrange("(t p) n -> p t n", p=P)
    xoth_r = xoth.rearrange("(t p) n -> p t n", p=P)
    wq_r = wqT.rearrange("(t p) o -> p t o", p=P)
    wk_r = wkT.rearrange("(t p) o -> p t o", p=P)
    wv_r = wvT.rearrange("(t p) o -> p t o", p=P)
    wp_r = wpT.rearrange("(t p) o -> p t o", p=P)
    wce1_r = wce1T.rearrange("(t p) h -> p t h", p=P)
    out_r = out.rearrange("(t p) n -> t p n", p=P)

    with tile.TileContext(nc) as tc:
        with (
            tc.tile_pool(name="cst", bufs=1) as cst,
            tc.tile_pool(name="ework", bufs=1) as ework,
            tc.tile_pool(name="work", bufs=2) as work,
            tc.tile_pool(name="ps", bufs=1, space="PSUM") as psum,
        ):
            _loop = ExitStack()
            if reps > 1:
                _loop.enter_context(tc.For_i(0, reps))
            # ---- loads ----
            wq_sb = cst.tile([P, CT, C], BF16)
            nc.sync.dma_start(wq_sb[:], wq_r[:])
            wk_sb = cst.tile([P, CT, C], BF16)
            nc.sync.dma_start(wk_sb[:], wk_r[:])
            wv_sb = cst.tile([P, CT, C], BF16)
            nc.sync.dma_start(wv_sb[:], wv_r[:])
            wp_sb = cst.tile([P, CT, C], BF16)
            nc.sync.dma_start(wp_sb[:], wp_r[:])
            wce1_sb = cst.tile([P, CT, HID], F32)
            nc.sync.dma_start(wce1_sb[:], wce1_r[:])
            wce2_sb = cst.tile([HID, 1], F32)
            nc.sync.dma_start(wce2_sb[:], wce2T[:])

            xq_sb = cst.tile([P, CT, QN], BF16)
            for j in range(QN // 512):
                for t in range(CT):
                    qeng = nc.scalar if (dma2 and t == 1) else nc.sync
                    qeng.dma_start(
                        xq_sb[:, t, ts(j, 512)], xq_r[:, t, ts(j, 512)]
                    )

            ones_bf = cst.tile([P, 1], BF16)
            nc.vector.memset(ones_bf[:], 1.0)
            onesrow = cst.tile([1, P], BF16)
            nc.vector.memset(onesrow[:], 1.0)

            # ---- stream x in chunks: cast to bf16 (ACT) + feat stats ----
            # keys are ordered [my half | other half]; attention is
            # permutation-invariant over keys so any consistent order works
            NXC = 8  # x chunks total per c-tile (4 per half)
            XCH = N // NXC
            xb_sb = cst.tile([P, CT, N], BF16)
            xres_tiles = {}
            xsum8 = cst.tile([P, CT, NXC], F32)
            xmax8 = cst.tile([P, CT, NXC], F32)
            feat = cst.tile([P, CT], F32)
            NHC = QN // XCH  # chunks per half
            for j in range(NXC):
                for t in range(CT):
                    if j < NHC:
                        xrt = work.tile([P, XCH], F32, tag="xres", bufs=2 * NHC)
                        xres_tiles[(t, j)] = xrt
                        xc = xrt[:]
                        xeng = nc.scalar if (dma2 and t == 1) else nc.sync
                        xeng.dma_start(xc, xres_r[:, t, ts(j, XCH)])
                    else:
                        xcw = work.tile([P, XCH], F32, tag="xc", bufs=2)
                        xc = xcw[:]
                        xeng = nc.scalar if (dma2 and t == 1) else nc.sync
                        xeng.dma_start(
                            xc, xoth_r[:, t, ts(j - NHC, XCH)]
                        )
                    nc.scalar.activation(
                        xb_sb[:, t, ts(j, XCH)], xc, AF.Copy,
                        accum_out=xsum8[:, t, j:j + 1],
                    )
                    getattr(nc, max_eng).reduce_max(
                        xmax8[:, t, j:j + 1], xc,
                        axis=mybir.AxisListType.X,
                    )
            for t in range(CT):
                # feat = sum(xsum8)/N + max(xmax8)
                xm = work.tile([P, 1], F32, tag="xm")
                nc.vector.reduce_max(
                    xm[:], xmax8[:, t, :], axis=mybir.AxisListType.X
                )
                xs = work.tile([P, 1], F32, tag="xs")
                nc.vector.reduce_sum(
                    xs[:], xsum8[:, t, :], axis=mybir.AxisListType.X
                )
                nc.vector.scalar_tensor_tensor(
                    feat[:, t:t + 1], xs[:], 1.0 / N, xm[:],
                    op0=ALU.mult, op1=ALU.add,
                )

            # ---- gate MLP (tiny) ----
            ph = psum.tile([HID, 1], F32, tag="pr", bufs=pr_bufs)
            for t in range(CT):
                nc.tensor.matmul(
                    ph[:], wce1_sb[:, t, :], feat[:, t:t + 1],
                    start=(t == 0), stop=(t == CT - 1),
                )
            sig = cst.tile([HID, 1], F32)
            nc.scalar.activation(sig[:], ph[:], AF.Sigmoid)
            hid_sb = cst.tile([HID, 1], F32)
            nc.vector.tensor_tensor(hid_sb[:], ph[:], sig[:], op=ALU.mult)
            pc = psum.tile([1, 1], F32, tag="pr", bufs=pr_bufs)
            nc.tensor.matmul(pc[:], wce2_sb[:], hid_sb[:])
            cmplx = cst.tile([1, 1], F32)
            nc.scalar.activation(cmplx[:], pc[:], AF.Sigmoid)

            # ---- Q, K (channel-major), V (token-major) ----
            k_sb = cst.tile([P, CT, N], BF16)
            q_sb = cst.tile([P, CT, QN], BF16)
            v_sb = cst.tile([P, MT, C], BF16)
            for j in range(QN // 512):
                for t in range(CT):
                    pq = psum.tile([P, 512], F32, tag="po", bufs=2)
                    for kc in range(CT):
                        nc.tensor.matmul(
                            pq[:], wq_sb[:, kc, ts(t, P)],
                            xq_sb[:, kc, ts(j, 512)],
                            start=(kc == 0), stop=(kc == CT - 1),
                        )
                    nc.vector.tensor_copy(q_sb[:, t, ts(j, 512)], pq[:])
            for j in range(N // 512):
                for t in range(CT):
                    pk = psum.tile([P, 512], F32, tag="po", bufs=2)
                    for kc in range(CT):
                        nc.tensor.matmul(
                            pk[:], wk_sb[:, kc, ts(t, P)],
                            xb_sb[:, kc, ts(j, 512)],
                            start=(kc == 0), stop=(kc == CT - 1),
                        )
                    nc.vector.tensor_copy(k_sb[:, t, ts(j, 512)], pk[:])

            for np_ in range(MT // 2):
                pv2 = psum.tile([P, 2, C], F32, tag="po", bufs=2)
                for h in range(2):
                    nt = 2 * np_ + h
                    for kc in range(CT):
                        nc.tensor.matmul(
                            pv2[:, h, :], xb_sb[:, kc, ts(nt, P)],
                            wv_sb[:, kc, :],
                            start=(kc == 0), stop=(kc == CT - 1),
                        )
                nc.scalar.activation(
                    v_sb[:, 2 * np_:2 * np_ + 2, :], pv2[:], AF.Copy
                )

            # ---- attention + per-chunk projection/residual ----
            o_sb = cst.tile([P, CT, QN], BF16)
            for qc in range(NQC if not no_attn else 0):
                et = ework.tile([P, MT, QCH], BF16, tag="exp", bufs=et_bufs)
                pr = psum.tile([1, QCH], F32, tag="pr", bufs=pr_bufs)
                # pairwise tree-sum of exp tiles on DVE (bf16 adds are
                # cheap and keep the PE free); one ones-matmul at the end
                # does the cross-partition reduction.
                flat_tree = "flattree" in variant
                tree_stack = []
                if flat_tree:
                    tree16 = work.tile([P, MT // 2, QCH], BF16, tag="tree16",
                                       bufs=2)

                def tree_push(leaf):
                    cur = (0, leaf)
                    while tree_stack and tree_stack[-1][0] == cur[0]:
                        lv, prev = tree_stack.pop()
                        dst = work.tile([P, QCH], BF16, tag="tree", bufs=7)
                        nc.vector.tensor_tensor(dst[:], prev, cur[1], op=ALU.add)
                        cur = (lv + 1, dst[:])
                    tree_stack.append(cur)

                if exp_pair:
                    G = 4 if exp_quad else 2
                    po_list = []
                    if not no_av:
                        for ct in range(CT):
                            po = psum.tile(
                                [P, QCH], F32, tag="po", bufs=2, name=f"po{ct}"
                            )
                            po_list.append(po)
                    for mp in range(MT // G):
                        pstg = psum.tile([P, G, QCH], F32, tag="ps2",
                                         bufs=(1 if exp_quad else 2))
                        for h in range(G):
                            mt = G * mp + h
                            for kc in range(CT):
                                nc.tensor.matmul(
                                    pstg[:, h, :], k_sb[:, kc, ts(mt, P)],
                                    q_sb[:, kc, ts(qc, QCH)],
                                    start=(kc == 0), stop=(kc == CT - 1),
                                )
                        nc.scalar.activation(
                            et[:, G * mp:G * mp + G, :], pstg[:], AF.Exp
                        )
                        for h in range(G):
                            mt = G * mp + h
                            for ct, po in enumerate(po_list):
                                nc.tensor.matmul(
                                    po[:], v_sb[:, mt, ts(ct, P)],
                                    et[:, mt, :],
                                    start=(mt == 0), stop=(mt == MT - 1),
                                    skip_group_check=True,
                                )
                        if not no_sum:
                            for h in range(0, G, 2):
                                tree_push(et[:, G * mp + h, :])
                                tree_push(et[:, G * mp + h + 1, :])
                else:
                    po_list = []
                    if not no_av:
                        for ct in range(CT):
                            po = psum.tile(
                                [P, QCH], F32, tag="po", bufs=2, name=f"po{ct}"
                            )
                            po_list.append(po)
                    for mt in range(MT):
                        pst = psum.tile([P, QCH], F32, tag="ps", bufs=5)
                        for kc in range(CT):
                            nc.tensor.matmul(
                                pst[:], k_sb[:, kc, ts(mt, P)],
                                q_sb[:, kc, ts(qc, QCH)],
                                start=(kc == 0), stop=(kc == CT - 1),
                            )
                        nc.scalar.activation(et[:, mt, :], pst[:], AF.Exp)
                        # AV for this m-tile rides right behind the exp so the
                        # PE hides the ACT latency instead of stalling on it
                        for ct, po in enumerate(po_list):
                            nc.tensor.matmul(
                                po[:], v_sb[:, mt, ts(ct, P)], et[:, mt, :],
                                start=(mt == 0), stop=(mt == MT - 1),
                                skip_group_check=True,
                            )
                        if not no_sum and mt % 2 == 1:
                            if flat_tree:
                                nc.vector.tensor_tensor(
                                    tree16[:, mt // 2, :], et[:, mt - 1, :],
                                    et[:, mt, :], op=ALU.add,
                                )
                            else:
                                tree_push(et[:, mt - 1, :])
                                tree_push(et[:, mt, :])
                if not no_sum:
                    if flat_tree:
                        # upper tree levels 16 -> 8 -> 4 -> 2 -> 1 in place
                        w_half = MT // 4
                        while w_half >= 1:
                            nc.vector.tensor_tensor(
                                tree16[:, :w_half, :], tree16[:, :w_half, :],
                                tree16[:, w_half:2 * w_half, :], op=ALU.add,
                            )
                            w_half //= 2
                        nc.tensor.matmul(pr[:], ones_bf[:], tree16[:, 0, :])
                    else:
                        assert len(tree_stack) == 1
                        nc.tensor.matmul(pr[:], ones_bf[:], tree_stack[-1][1])
                # 1/sum * gate, broadcast to 128 partitions via PE
                if not no_sum:
                    rr = work.tile([1, QCH], F32, tag="rr")
                    nc.vector.reciprocal(rr[:], pr[:])
                    rr2 = work.tile([1, QCH], BF16, tag="rr2")
                    nc.vector.tensor_scalar_mul(rr2[:], rr[:], cmplx[:1, :1])
                    pb = psum.tile([P, QCH], F32, tag="pr", bufs=pr_bufs)
                    nc.tensor.matmul(pb[:], onesrow[:], rr2[:])
                    recipB = work.tile([P, QCH], F32, tag="recipB", bufs=1)
                    nc.scalar.activation(recipB[:], pb[:], AF.Copy)
                for ct in range(CT if not no_av else 0):
                    po = po_list[ct]
                    # o = (po * recipB) -> bf16 (normalization + gate folded)
                    if no_sum:
                        nc.vector.tensor_copy(o_sb[:, ct, ts(qc, QCH)], po[:])
                    else:
                        nc.vector.tensor_tensor(
                            o_sb[:, ct, ts(qc, QCH)], po[:], recipB[:],
                            op=ALU.mult,
                        )
                # projection + residual for this chunk
                for ct in range(CT if not no_av else 0):
                    pp = psum.tile([P, QCH], F32, tag="po", bufs=2)
                    for kc in range(CT):
                        nc.tensor.matmul(
                            pp[:], wp_sb[:, kc, ts(ct, P)],
                            o_sb[:, kc, ts(qc, QCH)],
                            start=(kc == 0), stop=(kc == CT - 1),
                        )
                    outt = work.tile([P, QCH], F32, tag="outt", bufs=3)
                    nc.vector.tensor_tensor(
                        outt[:], pp[:], xres_tiles[(ct, qc)][:], op=ALU.add,
                    )
                    nc.sync.dma_start(out_r[ct, :, ts(qc, QCH)], outt[:])
            _loop.close()

    nc.finalize()
    return nc


_NC_CACHE = {}


def _get_nc():
    if "nc" not in _NC_CACHE:
        _NC_CACHE["nc"] = _build()
    return _NC_CACHE["nc"]


def kernel(x, w_ce1, w_ce2, wq, wk, wv, wproj):
    x = np.asarray(x, dtype=np.float32)
    assert x.shape == (B, C, H, W)

    wqT = np.ascontiguousarray(np.asarray(wq, np.float32).T * SCALE).astype(bf16)
    wkT = np.ascontiguousarray(np.asarray(wk, np.float32).T).astype(bf16)
    wvT = np.ascontiguousarray(np.asarray(wv, np.float32).T).astype(bf16)
    wpT = np.ascontiguousarray(np.asarray(wproj, np.float32).T).astype(bf16)
    wce1T = np.ascontiguousarray(np.asarray(w_ce1, np.float32).T)
    wce2T = np.ascontiguousarray(np.asarray(w_ce2, np.float32).T)

    in_maps = []
    for c in range(NCORES):
        b, h = divmod(c, NCORES // B)
        xf = np.ascontiguousarray(x[b].reshape(C, N))
        sl = slice(h * QN, (h + 1) * QN)
        slo = slice((1 - h) * QN, (2 - h) * QN)
        in_maps.append({
            "xq": np.ascontiguousarray(xf[:, sl]).astype(bf16),
            "xres": np.ascontiguousarray(xf[:, sl]),
            "xoth": np.ascontiguousarray(xf[:, slo]),
            "wqT": wqT, "wkT": wkT, "wvT": wvT, "wpT": wpT,
            "wce1T": wce1T, "wce2T": wce2T,
        })

    res = run_bass_kernel_spmd(_get_nc(), in_maps, list(range(NCORES)))

    out = np.empty((B, C, N), dtype=np.float32)
    for c in range(NCORES):
        b, h = divmod(c, NCORES // B)
        out[b][:, h * QN:(h + 1) * QN] = res.results[c]["out"]
    return out.reshape(B, C, H, W)
